# revision 26
# baseline (speedup 1.0000x reference)
"""Trainium2 Bass kernel for nn_MultiHeadModulator (8-core SPMD).

Math reformulation (exact): with a single query q = Wq@z_curr+bq,
  - dot scores:  score[l,h] = z[l]·A[:,h] + c[h],   A[:,h] = Wk[hb,:]^T @ q[hb]
  - rel scores fold into a per-(l,h) additive bias known on the host
  - value sum:   sum_l e[l,h]*v[l] = Wv @ (sum_l e[l,h]*z[l]) + (sum_l e[l,h])*bv
so the device only computes, per L-shard:
  score^T = A^T z^T   (PE, fp8 DoubleRow),  e^T = exp(scale*score + c_h) * fac
  U[h,:] += e^T z     (PE, fp8 DoubleRow),  S[h] from exp's accum_out
and the host applies Wv/Wo and the softmax normalization to the tiny [8,512]
all-core sums.  Softmax runs without max-subtraction: scores are O(1) by
construction (validated |score| < 3).

Sharding: z_past split into 8 contiguous shards of 8192 rows, one per core.
The host ships each shard twice (feature-major for scores, row-major for U)
in fp8, pre-packed for DoubleRow access patterns.

Scheduling notes (hard-won, via perfetto):
  - ALL bulk loads ride the sync HWDGE ring as ~10 large D2D triggers.
    Measured: the sync ring alone sustains ~420GB/s (8.4MB lands by
    ~30us); fanning bulk out to the scalar HWDGE ring (~35-55GB/s) or
    gpsimd SWDGE (~115GB/s, ~11us boot) actively STEALS sync throughput
    (sync dropped 8x while SWDGE streamed) - every multi-ring variant
    measured 6+us slower end-to-end.
  - the PE has a DVFS p-state ramp (0.65 -> 1.2 -> 2.4GHz after 3us of
    continuous execution); steady measured matmuls (379ns per 512-col
    fp8-DR stream) sit at the LOW state because per-block stalls keep
    resetting the ramp.  The loop is software-pipelined TWO deep
    (scores b+1 | transposes b | U-matmul b-1) so the DVE e8-cast
    between transpose and U of a block never blocks the PE stream.
  - weight-side DoubleRow LDWEIGHTS requires the pair-dim step to be a
    multiple of 16 elements (a_dr lives in zt0 cols 512:528; 528=33*16).
  - nc.vector.tensor_tensor_reduce crashes on HW (fine in CoreSim); S uses
    the exp's accum_out for uncorrected blocks + a DVE reduce for block 0.
  - cb ships as bf16 inside the const tile (a per-head-constant bias error
    cancels in the U/S softmax ratio).
  - PSUM budget (8 banks): 4x score + 3x e-transpose + 1x U accumulator.
"""

import numpy as np
import ml_dtypes

import concourse.bass as bass  # noqa: F401  (engine namespaces live on the nc)
import concourse.mybir as mybir
import concourse.tile as tile
from concourse import bacc
from concourse.bass_utils import run_bass_kernel_spmd

HEADS = 8
REL_MAX = 64
DIM = 256
D2 = 512                      # flattened real feature dim
HD = DIM // HEADS             # 32 complex => 64 reals per head block
L_TOTAL = 65536
N_CORES = 8
L_SHARD = L_TOTAL // N_CORES  # 8192
N_BLOCKS = L_SHARD // 512     # 16 blocks of 512 rows
BLK_PER_SUPER = 2
N_SUPER = 6                   # supers cover blocks 4-15
SCALE = 1.0 / np.sqrt(HD)

FP8 = ml_dtypes.float8_e4m3   # == mybir.dt.float8e4 (trainium E4M3, max 240)
BF16 = ml_dtypes.bfloat16

TRACE = False                 # test.py can flip this for profiling runs
TRACE_KW = {}

_cached = {}


def _build_program(full_fac: bool):
    nc = bacc.Bacc(
        "TRN2", target_bir_lowering=False, debug=False, num_devices=N_CORES
    )
    DR = mybir.MatmulPerfMode.DoubleRow
    f8 = mybir.dt.float8e4
    facw = L_SHARD if full_fac else 512

    # block 0 of zt with a_dr packed into cols 512:528 of the last axis
    ZT0 = nc.dram_tensor("zt0", [128, 2, 2, 528], f8, kind="ExternalInput")
    ZT1 = nc.dram_tensor("zt1", [128, 3, 2, 2, 512], f8, kind="ExternalInput")
    ZN1 = nc.dram_tensor("zn1", [128, 4, 2, 2, 512], f8, kind="ExternalInput")
    ZTS = nc.dram_tensor(
        "zts", [N_SUPER, 128, BLK_PER_SUPER, 2, 2, 512], f8,
        kind="ExternalInput",
    )
    ZNS = nc.dram_tensor(
        "zns", [N_SUPER, 128, BLK_PER_SUPER, 2, 2, 512], f8,
        kind="ExternalInput",
    )
    # col 0: cb (bf16; per-head-constant error cancels in U/S), 1:9 identity,
    # 9:9+facw rel-bias correction factors
    CST = nc.dram_tensor("cst", [8, 9 + facw], mybir.dt.bfloat16,
                         kind="ExternalInput")
    OUT_U = nc.dram_tensor("out_u", [8, 512], mybir.dt.float32,
                           kind="ExternalOutput")
    OUT_S = nc.dram_tensor("out_s", [8, N_BLOCKS], mybir.dt.float32,
                           kind="ExternalOutput")

    with tile.TileContext(nc) as tc:
        with (
            tc.tile_pool(name="zt0", bufs=1) as zt0_pool,
            tc.tile_pool(name="zbig", bufs=1) as zbig_pool,
            tc.tile_pool(name="consts", bufs=1) as const_pool,
            tc.tile_pool(name="et", bufs=8) as et_pool,
            tc.tile_pool(name="e8", bufs=8) as e8_pool,
            tc.tile_pool(name="outs", bufs=1) as out_pool,
            tc.tile_pool(name="ps_sc", bufs=4, space="PSUM") as sc_pool,
            tc.tile_pool(name="ps_etp", bufs=3, space="PSUM") as etp_pool,
            tc.tile_pool(name="ps_acc", bufs=1, space="PSUM") as acc_pool,
        ):
            zt0_sb = zt0_pool.tile([128, 2, 2, 528], f8)
            nc.sync.dma_start(zt0_sb[:], ZT0[:])
            cst_sb = const_pool.tile([8, 9 + facw], mybir.dt.bfloat16)
            nc.sync.dma_start(cst_sb[:], CST[:])
            zt1_sb = zbig_pool.tile([128, 3, 2, 2, 512], f8)
            nc.sync.dma_start(zt1_sb[:], ZT1[:])
            zn1_sb = zbig_pool.tile([128, 4, 2, 2, 512], f8)
            nc.sync.dma_start(zn1_sb[:], ZN1[:])
            zts_tiles = [None] * N_SUPER
            zns_tiles = [None] * N_SUPER
            for s in range(N_SUPER):
                zts_tiles[s] = zbig_pool.tile(
                    [128, BLK_PER_SUPER, 2, 2, 512], f8, name=f"zts_{s}"
                )
                nc.sync.dma_start(zts_tiles[s][:], ZTS[s])
                zns_tiles[s] = zbig_pool.tile(
                    [128, BLK_PER_SUPER, 2, 2, 512], f8, name=f"zns_{s}"
                )
                nc.sync.dma_start(zns_tiles[s][:], ZNS[s])

            u_ps = acc_pool.tile([8, 512], mybir.dt.float32)
            outs_sb = out_pool.tile([8, N_BLOCKS], mybir.dt.float32)
            u_sb = out_pool.tile([8, 512], mybir.dt.float32)

            def zt_view(b):
                if b == 0:
                    return zt0_sb[:, :, :, 0:512]
                if b < 4:
                    return zt1_sb[:, b - 1]
                return zts_tiles[(b - 4) // BLK_PER_SUPER][
                    :, (b - 4) % BLK_PER_SUPER
                ]

            def zn_view(b):
                if b < 4:
                    return zn1_sb[:, b]
                return zns_tiles[(b - 4) // BLK_PER_SUPER][
                    :, (b - 4) % BLK_PER_SUPER
                ]

            def scores(b):
                # score^T[h, l]: 512 rows, K=512 via 2x fp8 DoubleRow passes
                zt_t = zt_view(b)
                sc = sc_pool.tile(
                    [8, 512], mybir.dt.float32, tag="sc", name=f"sc_{b}"
                )
                for cpair in range(2):
                    nc.tensor.matmul(
                        sc[:],
                        zt0_sb[:, cpair, :, 512:520],
                        zt_t[:, cpair] if b else zt0_sb[:, cpair, :, 0:512],
                        start=(cpair == 0),
                        stop=(cpair == 1),
                        perf_mode=DR,
                    )
                et = et_pool.tile(
                    [8, 512], mybir.dt.bfloat16, tag="et", name=f"et_{b}"
                )
                # for fac==1 blocks, S comes free from the exp's accum_out
                accum = (
                    {}
                    if (full_fac or b == 0)
                    else {"accum_out": outs_sb[:, b : b + 1]}
                )
                nc.scalar.activation(
                    et[:],
                    sc[:],
                    mybir.ActivationFunctionType.Exp,
                    bias=cst_sb[:, 0:1],
                    scale=float(SCALE),
                    **accum,
                )
                # rel-bias correction factors: only block 0 deviates from 1
                # in the common curr_pos regime (full_fac covers the rest)
                if full_fac or b == 0:
                    etc = et_pool.tile(
                        [8, 512], mybir.dt.bfloat16, tag="etc", name=f"etc_{b}"
                    )
                    nc.vector.tensor_mul(
                        etc[:], et[:], cst_sb[:, 9 + 512 * b : 9 + 512 * (b + 1)]
                    )
                    # S for corrected blocks: one DVE free-axis reduction
                    nc.vector.tensor_reduce(
                        outs_sb[:, b : b + 1],
                        etc[:],
                        axis=mybir.AxisListType.X,
                        op=mybir.AluOpType.add,
                    )
                else:
                    etc = et
                return etc

            def transposes(b, etc):
                # transpose e^T -> e[l,h] in 4x [8,128] chunks (PE+identity)
                etp = etp_pool.tile(
                    [128, 4, 8], mybir.dt.bfloat16, tag="etp", name=f"etp_{b}"
                )
                for quad in range(4):
                    nc.tensor.transpose(
                        etp[:, quad],
                        etc[:, 128 * quad : 128 * (quad + 1)],
                        cst_sb[:, 1:9],
                    )
                e8 = e8_pool.tile([128, 4, 16], f8, tag="e8", name=f"e8_{b}")
                nc.vector.tensor_copy(e8[:, :, 0:8], etp[:])
                return e8

            def weighted_sum(b, e8, first, last):
                zn_t = zn_view(b)
                for s in range(2):
                    nc.tensor.matmul(
                        u_ps[:],
                        e8[:, 2 * s : 2 * s + 2, 0:8],
                        zn_t[:, s],
                        start=(first and s == 0),
                        stop=(last and s == 1),
                        perf_mode=DR,
                    )

            # two-deep software pipeline: the PE stream is
            #   [scores b+1 | transposes b | U-matmul b-1]
            # so exp(b) hides under scores(b+1) and the DVE e8-cast of
            # block b hides under scores(b+2)+transposes(b+1); the PE never
            # waits mid-block, which also keeps its DVFS p-state ramped
            pend = []  # [(block, etc-or-e8)]
            for b in range(N_BLOCKS):
                etc = scores(b)
                if pend:
                    pend[-1] = (pend[-1][0], transposes(*pend[-1]))
                if len(pend) == 2:
                    bu, e8_u = pend.pop(0)
                    weighted_sum(bu, e8_u, bu == 0, False)
                pend.append((b, etc))
            pend[-1] = (pend[-1][0], transposes(*pend[-1]))
            bu, e8_u = pend.pop(0)
            weighted_sum(bu, e8_u, bu == 0, False)
            bu, e8_u = pend.pop(0)
            weighted_sum(bu, e8_u, False, True)

            # S partials ride the idle sync ring; ACT (closest to PSUM,
            # free after the last exp) copies U, then its ring sends it
            nc.sync.dma_start(OUT_S[:], outs_sb[:])
            nc.scalar.copy(u_sb[:], u_ps[:])
            nc.sync.dma_start(OUT_U[:], u_sb[:])

    nc.compile()
    return nc


def _get_program(full_fac: bool):
    if full_fac not in _cached:
        _cached[full_fac] = _build_program(full_fac)
    return _cached[full_fac]


def kernel(curr_pos, z_curr, z_past, Wq, bq, Wk, bk, Wv, bv, Wo, bo, rel_bias):
    curr_pos = int(np.asarray(curr_pos))
    z_curr = np.asarray(z_curr, dtype=np.float32)
    z_past = np.asarray(z_past, dtype=np.float32)
    Wq = np.asarray(Wq, dtype=np.float32)
    bq = np.asarray(bq, dtype=np.float32)
    Wk = np.asarray(Wk, dtype=np.float32)
    bk = np.asarray(bk, dtype=np.float32)
    Wv = np.asarray(Wv, dtype=np.float32)
    bv = np.asarray(bv, dtype=np.float32)
    Wo = np.asarray(Wo, dtype=np.float32)
    bo = np.asarray(bo, dtype=np.float32)
    rel_bias = np.asarray(rel_bias, dtype=np.float32)

    # ---- host-side O(D^2) prep (f64) ----
    q = z_curr.reshape(-1).astype(np.float64) @ Wq.T.astype(np.float64) + bq
    A = np.zeros((D2, HEADS), np.float64)
    c = np.zeros(HEADS, np.float64)
    for h in range(HEADS):
        sl = slice(h * 2 * HD, (h + 1) * 2 * HD)
        A[:, h] = Wk[sl, :].T.astype(np.float64) @ q[sl]
        c[h] = bk[sl].astype(np.float64) @ q[sl]
    relflat = rel_bias.reshape(2 * REL_MAX + 1, D2).astype(np.float64)
    rb = np.stack(
        [
            relflat[:, h * 2 * HD : (h + 1) * 2 * HD] @ q[h * 2 * HD : (h + 1) * 2 * HD]
            for h in range(HEADS)
        ],
        axis=1,
    )  # [129, 8]
    idx = np.clip(
        curr_pos - L_TOTAL + np.arange(L_TOTAL) + REL_MAX, 0, 2 * REL_MAX
    ).astype(np.int64)

    z8 = np.clip(z_past.reshape(L_TOTAL, D2), -240.0, 240.0).astype(FP8)
    A8 = np.clip(A, -240.0, 240.0).astype(np.float32).astype(FP8)
    a_dr = np.zeros((128, 2, 2, 16), FP8)
    a_dr[:, :, :, 0:8] = A8.reshape(2, 2, 128, HEADS).transpose(2, 0, 1, 3)

    in_maps = []
    facs = []
    for core in range(N_CORES):
        zc = z8[core * L_SHARD : (core + 1) * L_SHARD]
        # zt_b[p, cpair, d, l] = zc[512*b + l, 256*cpair + 128*d + p]
        zt_all = np.ascontiguousarray(
            zc.reshape(N_BLOCKS, 512, 2, 2, 128).transpose(0, 4, 2, 3, 1)
        )
        # zn_b[p, s, d, f] = zc[512*b + 256*s + 128*d + p, f]
        zn_all = np.ascontiguousarray(
            zc.reshape(N_BLOCKS, 2, 2, 128, 512).transpose(0, 3, 1, 2, 4)
        )
        zt0 = np.concatenate([zt_all[0], a_dr], axis=3)

        def pk(blob, lo, hi):  # [nblk,128,2,2,512] -> [128,nblk,2,2,512]
            return np.ascontiguousarray(
                blob[lo:hi].transpose(1, 0, 2, 3, 4)
            )

        def pks(blob, lo, hi, per):  # -> [nsup,128,per,2,2,512]
            n = (hi - lo) // per
            return np.ascontiguousarray(
                blob[lo:hi].reshape(n, per, 128, 2, 2, 512).transpose(
                    0, 2, 1, 3, 4, 5
                )
            )

        idx_c = idx[core * L_SHARD : (core + 1) * L_SHARD]
        base = int(np.bincount(idx_c, minlength=2 * REL_MAX + 1).argmax())
        cb = ((c + rb[base]) * SCALE).astype(np.float32).reshape(HEADS, 1)
        fac = np.ascontiguousarray(
            np.exp((rb[idx_c] - rb[base]) * SCALE).T.astype(BF16)
        )
        facs.append(fac)
        in_maps.append(
            {
                "zt0": zt0,
                "zt1": pk(zt_all, 1, 4),
                "zn1": pk(zn_all, 0, 4),
                "zts": pks(zt_all, 4, 16, BLK_PER_SUPER),
                "zns": pks(zn_all, 4, 16, BLK_PER_SUPER),
                "cb": cb,
            }
        )

    # fast path: correction factors are 1.0 outside block 0 on every core
    full_fac = any(
        not np.all(f[:, 512:] == np.asarray(1.0, BF16)) for f in facs
    )
    facw = L_SHARD if full_fac else 512
    for core, m in enumerate(in_maps):
        cst = np.zeros((8, 9 + facw), BF16)
        cst[:, 0:1] = m.pop("cb").astype(BF16)
        cst[:, 1:9] = np.eye(8, dtype=BF16)
        cst[:, 9:] = facs[core][:, 0:facw]
        m["cst"] = cst

    nc = _get_program(full_fac)
    res = run_bass_kernel_spmd(
        nc, in_maps, list(range(N_CORES)), trace=TRACE, **TRACE_KW
    )
    if TRACE:
        kernel.last_result = res

    U = np.zeros((HEADS, D2), np.float64)
    S = np.zeros(HEADS, np.float64)
    for r in res.results:
        U += np.asarray(r["out_u"], dtype=np.float64)
        S += np.asarray(r["out_s"], dtype=np.float64).sum(axis=1)

    hvec = np.zeros(D2, np.float64)
    for h in range(HEADS):
        sl = slice(h * 2 * HD, (h + 1) * 2 * HD)
        hvec[sl] = Wv[sl, :].astype(np.float64) @ (U[h] / S[h]) + bv[sl]
    out = hvec @ Wo.T.astype(np.float64) + bo
    return out.reshape(DIM, 2).astype(np.float32)


# revision 27
# speedup vs baseline: 1.0986x; 1.0986x over previous
"""Trainium2 Bass kernel for nn_MultiHeadModulator (8-core SPMD).

Math reformulation (exact): with a single query q = Wq@z_curr+bq,
  - dot scores:  score[l,h] = z[l]·A[:,h] + c[h],   A[:,h] = Wk[hb,:]^T @ q[hb]
  - rel scores fold into a per-(l,h) additive bias known on the host
  - value sum:   sum_l e[l,h]*v[l] = Wv @ (sum_l e[l,h]*z[l]) + (sum_l e[l,h])*bv
so the device only computes, per L-shard:
  score^T = A^T z^T   (PE, fp8 DoubleRow),  e^T = exp(scale*score + c_h) * fac
  U[h,:] += e^T z     (PE, fp8 DoubleRow),  S[h] from exp's accum_out
and the host applies Wv/Wo and the softmax normalization to the tiny [8,512]
all-core sums.  Softmax runs without max-subtraction: scores are O(1) by
construction (validated |score| < 3).

Sharding: z_past split into 8 contiguous shards of 8192 rows, one per core.
The host ships each shard twice (feature-major for scores, row-major for U)
in fp8, pre-packed for DoubleRow access patterns.

Scheduling notes (hard-won, via perfetto):
  - ALL bulk loads ride the sync HWDGE ring as ~10 large D2D triggers.
    Measured: the sync ring alone sustains ~420GB/s (8.4MB lands by
    ~30us); fanning bulk out to the scalar HWDGE ring (~35-55GB/s) or
    gpsimd SWDGE (~115GB/s, ~11us boot) actively STEALS sync throughput
    (sync dropped 8x while SWDGE streamed) - every multi-ring variant
    measured 6+us slower end-to-end.
  - the PE has a DVFS p-state ramp (0.65 -> 1.2 -> 2.4GHz after 3us of
    continuous execution); steady measured matmuls (379ns per 512-col
    fp8-DR stream) sit at the LOW state because per-block stalls keep
    resetting the ramp.  The loop is software-pipelined TWO deep
    (scores b+1 | transposes b | U-matmul b-1) so the DVE e8-cast
    between transpose and U of a block never blocks the PE stream.
  - weight-side DoubleRow LDWEIGHTS requires the pair-dim step to be a
    multiple of 16 elements (a_dr lives in zt0 cols 512:528; 528=33*16).
  - nc.vector.tensor_tensor_reduce crashes on HW (fine in CoreSim); S uses
    the exp's accum_out for uncorrected blocks + a DVE reduce for block 0.
  - cb ships as bf16 inside the const tile (a per-head-constant bias error
    cancels in the U/S softmax ratio).
  - PSUM budget (8 banks): 4x score + 3x e-transpose + 1x U accumulator.
"""

import numpy as np
import ml_dtypes

import concourse.bass as bass  # noqa: F401  (engine namespaces live on the nc)
import concourse.mybir as mybir
import concourse.tile as tile
from concourse import bacc
from concourse.bass_utils import run_bass_kernel_spmd

HEADS = 8
REL_MAX = 64
DIM = 256
D2 = 512                      # flattened real feature dim
HD = DIM // HEADS             # 32 complex => 64 reals per head block
L_TOTAL = 65536
N_CORES = 8
L_SHARD = L_TOTAL // N_CORES  # 8192
N_BLOCKS = L_SHARD // 512     # 16 blocks of 512 rows
BLK_PER_SUPER = 4
N_SUPER = 3                   # supers cover blocks 4-15
SCALE = 1.0 / np.sqrt(HD)

FP8 = ml_dtypes.float8_e4m3   # == mybir.dt.float8e4 (trainium E4M3, max 240)
BF16 = ml_dtypes.bfloat16

TRACE = False                 # test.py can flip this for profiling runs
TRACE_KW = {}

_cached = {}


def _build_program(full_fac: bool):
    nc = bacc.Bacc(
        "TRN2", target_bir_lowering=False, debug=False, num_devices=N_CORES
    )
    DR = mybir.MatmulPerfMode.DoubleRow
    f8 = mybir.dt.float8e4
    facw = L_SHARD if full_fac else 512

    # block 0 of zt with a_dr packed into cols 512:528 of the last axis
    ZT0 = nc.dram_tensor("zt0", [128, 2, 2, 528], f8, kind="ExternalInput")
    ZT1 = nc.dram_tensor("zt1", [128, 3, 2, 2, 512], f8, kind="ExternalInput")
    ZN1 = nc.dram_tensor("zn1", [128, 4, 2, 2, 512], f8, kind="ExternalInput")
    ZTS = nc.dram_tensor(
        "zts", [N_SUPER, 128, BLK_PER_SUPER, 2, 2, 512], f8,
        kind="ExternalInput",
    )
    ZNS = nc.dram_tensor(
        "zns", [N_SUPER, 128, BLK_PER_SUPER, 2, 2, 512], f8,
        kind="ExternalInput",
    )
    # col 0: cb (bf16; per-head-constant error cancels in U/S), 1:9 identity,
    # 9:9+facw rel-bias correction factors
    CST = nc.dram_tensor("cst", [8, 9 + facw], mybir.dt.bfloat16,
                         kind="ExternalInput")
    OUT_U = nc.dram_tensor("out_u", [8, 512], mybir.dt.float32,
                           kind="ExternalOutput")
    OUT_S = nc.dram_tensor("out_s", [8, N_BLOCKS], mybir.dt.float32,
                           kind="ExternalOutput")

    with tile.TileContext(nc) as tc:
        with (
            tc.tile_pool(name="zt0", bufs=1) as zt0_pool,
            tc.tile_pool(name="zbig", bufs=1) as zbig_pool,
            tc.tile_pool(name="consts", bufs=1) as const_pool,
            tc.tile_pool(name="et", bufs=8) as et_pool,
            tc.tile_pool(name="e8", bufs=8) as e8_pool,
            tc.tile_pool(name="outs", bufs=1) as out_pool,
            tc.tile_pool(name="ps_sc", bufs=5, space="PSUM") as sc_pool,
            tc.tile_pool(name="ps_etp", bufs=2, space="PSUM") as etp_pool,
            tc.tile_pool(name="ps_acc", bufs=1, space="PSUM") as acc_pool,
        ):
            zt0_sb = zt0_pool.tile([128, 2, 2, 528], f8)
            nc.sync.dma_start(zt0_sb[:], ZT0[:])
            cst_sb = const_pool.tile([8, 9 + facw], mybir.dt.bfloat16)
            nc.sync.dma_start(cst_sb[:], CST[:])
            zt1_sb = zbig_pool.tile([128, 3, 2, 2, 512], f8)
            nc.sync.dma_start(zt1_sb[:], ZT1[:])
            zn1_sb = zbig_pool.tile([128, 4, 2, 2, 512], f8)
            nc.sync.dma_start(zn1_sb[:], ZN1[:])
            zts_tiles = [None] * N_SUPER
            zns_tiles = [None] * N_SUPER
            for s in range(N_SUPER):
                zts_tiles[s] = zbig_pool.tile(
                    [128, BLK_PER_SUPER, 2, 2, 512], f8, name=f"zts_{s}"
                )
                nc.sync.dma_start(zts_tiles[s][:], ZTS[s])
                zns_tiles[s] = zbig_pool.tile(
                    [128, BLK_PER_SUPER, 2, 2, 512], f8, name=f"zns_{s}"
                )
                nc.sync.dma_start(zns_tiles[s][:], ZNS[s])

            u_ps = acc_pool.tile([8, 512], mybir.dt.float32)
            outs_sb = out_pool.tile([8, N_BLOCKS], mybir.dt.float32)
            u_sb = out_pool.tile([8, 512], mybir.dt.float32)

            def zt_view(b):
                if b == 0:
                    return zt0_sb[:, :, :, 0:512]
                if b < 4:
                    return zt1_sb[:, b - 1]
                return zts_tiles[(b - 4) // BLK_PER_SUPER][
                    :, (b - 4) % BLK_PER_SUPER
                ]

            def zn_view(b):
                if b < 4:
                    return zn1_sb[:, b]
                return zns_tiles[(b - 4) // BLK_PER_SUPER][
                    :, (b - 4) % BLK_PER_SUPER
                ]

            def scores(b):
                # score^T[h, l]: 512 rows, K=512 via 2x fp8 DoubleRow passes
                zt_t = zt_view(b)
                sc = sc_pool.tile(
                    [8, 512], mybir.dt.float32, tag="sc", name=f"sc_{b}"
                )
                for cpair in range(2):
                    nc.tensor.matmul(
                        sc[:],
                        zt0_sb[:, cpair, :, 512:520],
                        zt_t[:, cpair] if b else zt0_sb[:, cpair, :, 0:512],
                        start=(cpair == 0),
                        stop=(cpair == 1),
                        perf_mode=DR,
                    )
                et = et_pool.tile(
                    [8, 512], mybir.dt.bfloat16, tag="et", name=f"et_{b}"
                )
                nc.scalar.activation(
                    et[:],
                    sc[:],
                    mybir.ActivationFunctionType.Exp,
                    bias=cst_sb[:, 0:1],
                    scale=float(SCALE),
                )
                # rel-bias correction factors: only block 0 deviates from 1
                # in the common curr_pos regime (full_fac covers the rest)
                if full_fac or b == 0:
                    etc = et_pool.tile(
                        [8, 512], mybir.dt.bfloat16, tag="etc", name=f"etc_{b}"
                    )
                    nc.vector.tensor_mul(
                        etc[:], et[:], cst_sb[:, 9 + 512 * b : 9 + 512 * (b + 1)]
                    )
                else:
                    etc = et
                # S on the idle DVE (keeps the ACT chain pure exp: no
                # 185ns ACTIVATION_READ_ACCUMULATOR serializing it)
                nc.vector.tensor_reduce(
                    outs_sb[:, b : b + 1],
                    etc[:],
                    axis=mybir.AxisListType.X,
                    op=mybir.AluOpType.add,
                )
                return etc

            def transposes(b, etc):
                # transpose e^T -> e[l,h] in 4x [8,128] chunks (PE+identity)
                etp = etp_pool.tile(
                    [128, 4, 8], mybir.dt.bfloat16, tag="etp", name=f"etp_{b}"
                )
                for quad in range(4):
                    nc.tensor.transpose(
                        etp[:, quad],
                        etc[:, 128 * quad : 128 * (quad + 1)],
                        cst_sb[:, 1:9],
                    )
                e8 = e8_pool.tile([128, 4, 16], f8, tag="e8", name=f"e8_{b}")
                nc.vector.tensor_copy(e8[:, :, 0:8], etp[:])
                return e8

            def weighted_sum(b, e8, first, last):
                zn_t = zn_view(b)
                for s in range(2):
                    nc.tensor.matmul(
                        u_ps[:],
                        e8[:, 2 * s : 2 * s + 2, 0:8],
                        zn_t[:, s],
                        start=(first and s == 0),
                        stop=(last and s == 1),
                        perf_mode=DR,
                    )

            # two-deep software pipeline: the PE stream is
            #   [scores b+1 | transposes b | U-matmul b-1]
            # so exp(b) hides under scores(b+1) and the DVE e8-cast of
            # block b hides under scores(b+2)+transposes(b+1); the PE never
            # waits mid-block, which also keeps its DVFS p-state ramped
            pend = []  # [(block, etc-or-e8)]
            for b in range(N_BLOCKS):
                etc = scores(b)
                if pend:
                    pend[-1] = (pend[-1][0], transposes(*pend[-1]))
                if len(pend) == 2:
                    bu, e8_u = pend.pop(0)
                    weighted_sum(bu, e8_u, bu == 0, False)
                pend.append((b, etc))
            pend[-1] = (pend[-1][0], transposes(*pend[-1]))
            bu, e8_u = pend.pop(0)
            weighted_sum(bu, e8_u, bu == 0, False)
            bu, e8_u = pend.pop(0)
            weighted_sum(bu, e8_u, False, True)

            # S partials ride the idle sync ring; ACT (closest to PSUM,
            # free after the last exp) copies U, then its ring sends it
            nc.sync.dma_start(OUT_S[:], outs_sb[:])
            nc.scalar.copy(u_sb[:], u_ps[:])
            nc.sync.dma_start(OUT_U[:], u_sb[:])

    nc.compile()
    return nc


def _get_program(full_fac: bool):
    if full_fac not in _cached:
        _cached[full_fac] = _build_program(full_fac)
    return _cached[full_fac]


def kernel(curr_pos, z_curr, z_past, Wq, bq, Wk, bk, Wv, bv, Wo, bo, rel_bias):
    curr_pos = int(np.asarray(curr_pos))
    z_curr = np.asarray(z_curr, dtype=np.float32)
    z_past = np.asarray(z_past, dtype=np.float32)
    Wq = np.asarray(Wq, dtype=np.float32)
    bq = np.asarray(bq, dtype=np.float32)
    Wk = np.asarray(Wk, dtype=np.float32)
    bk = np.asarray(bk, dtype=np.float32)
    Wv = np.asarray(Wv, dtype=np.float32)
    bv = np.asarray(bv, dtype=np.float32)
    Wo = np.asarray(Wo, dtype=np.float32)
    bo = np.asarray(bo, dtype=np.float32)
    rel_bias = np.asarray(rel_bias, dtype=np.float32)

    # ---- host-side O(D^2) prep (f64) ----
    q = z_curr.reshape(-1).astype(np.float64) @ Wq.T.astype(np.float64) + bq
    A = np.zeros((D2, HEADS), np.float64)
    c = np.zeros(HEADS, np.float64)
    for h in range(HEADS):
        sl = slice(h * 2 * HD, (h + 1) * 2 * HD)
        A[:, h] = Wk[sl, :].T.astype(np.float64) @ q[sl]
        c[h] = bk[sl].astype(np.float64) @ q[sl]
    relflat = rel_bias.reshape(2 * REL_MAX + 1, D2).astype(np.float64)
    rb = np.stack(
        [
            relflat[:, h * 2 * HD : (h + 1) * 2 * HD] @ q[h * 2 * HD : (h + 1) * 2 * HD]
            for h in range(HEADS)
        ],
        axis=1,
    )  # [129, 8]
    idx = np.clip(
        curr_pos - L_TOTAL + np.arange(L_TOTAL) + REL_MAX, 0, 2 * REL_MAX
    ).astype(np.int64)

    z8 = np.clip(z_past.reshape(L_TOTAL, D2), -240.0, 240.0).astype(FP8)
    A8 = np.clip(A, -240.0, 240.0).astype(np.float32).astype(FP8)
    a_dr = np.zeros((128, 2, 2, 16), FP8)
    a_dr[:, :, :, 0:8] = A8.reshape(2, 2, 128, HEADS).transpose(2, 0, 1, 3)

    in_maps = []
    facs = []
    for core in range(N_CORES):
        zc = z8[core * L_SHARD : (core + 1) * L_SHARD]
        # zt_b[p, cpair, d, l] = zc[512*b + l, 256*cpair + 128*d + p]
        zt_all = np.ascontiguousarray(
            zc.reshape(N_BLOCKS, 512, 2, 2, 128).transpose(0, 4, 2, 3, 1)
        )
        # zn_b[p, s, d, f] = zc[512*b + 256*s + 128*d + p, f]
        zn_all = np.ascontiguousarray(
            zc.reshape(N_BLOCKS, 2, 2, 128, 512).transpose(0, 3, 1, 2, 4)
        )
        zt0 = np.concatenate([zt_all[0], a_dr], axis=3)

        def pk(blob, lo, hi):  # [nblk,128,2,2,512] -> [128,nblk,2,2,512]
            return np.ascontiguousarray(
                blob[lo:hi].transpose(1, 0, 2, 3, 4)
            )

        def pks(blob, lo, hi, per):  # -> [nsup,128,per,2,2,512]
            n = (hi - lo) // per
            return np.ascontiguousarray(
                blob[lo:hi].reshape(n, per, 128, 2, 2, 512).transpose(
                    0, 2, 1, 3, 4, 5
                )
            )

        idx_c = idx[core * L_SHARD : (core + 1) * L_SHARD]
        base = int(np.bincount(idx_c, minlength=2 * REL_MAX + 1).argmax())
        cb = ((c + rb[base]) * SCALE).astype(np.float32).reshape(HEADS, 1)
        fac = np.ascontiguousarray(
            np.exp((rb[idx_c] - rb[base]) * SCALE).T.astype(BF16)
        )
        facs.append(fac)
        in_maps.append(
            {
                "zt0": zt0,
                "zt1": pk(zt_all, 1, 4),
                "zn1": pk(zn_all, 0, 4),
                "zts": pks(zt_all, 4, 16, BLK_PER_SUPER),
                "zns": pks(zn_all, 4, 16, BLK_PER_SUPER),
                "cb": cb,
            }
        )

    # fast path: correction factors are 1.0 outside block 0 on every core
    full_fac = any(
        not np.all(f[:, 512:] == np.asarray(1.0, BF16)) for f in facs
    )
    facw = L_SHARD if full_fac else 512
    for core, m in enumerate(in_maps):
        cst = np.zeros((8, 9 + facw), BF16)
        cst[:, 0:1] = m.pop("cb").astype(BF16)
        cst[:, 1:9] = np.eye(8, dtype=BF16)
        cst[:, 9:] = facs[core][:, 0:facw]
        m["cst"] = cst

    nc = _get_program(full_fac)
    res = run_bass_kernel_spmd(
        nc, in_maps, list(range(N_CORES)), trace=TRACE, **TRACE_KW
    )
    if TRACE:
        kernel.last_result = res

    U = np.zeros((HEADS, D2), np.float64)
    S = np.zeros(HEADS, np.float64)
    for r in res.results:
        U += np.asarray(r["out_u"], dtype=np.float64)
        S += np.asarray(r["out_s"], dtype=np.float64).sum(axis=1)

    hvec = np.zeros(D2, np.float64)
    for h in range(HEADS):
        sl = slice(h * 2 * HD, (h + 1) * 2 * HD)
        hvec[sl] = Wv[sl, :].astype(np.float64) @ (U[h] / S[h]) + bv[sl]
    out = hvec @ Wo.T.astype(np.float64) + bo
    return out.reshape(DIM, 2).astype(np.float32)
